# revision 26
# baseline (speedup 1.0000x reference)
"""Trainium2 Bass kernel for nn_CIND_Block (cin_diff + 3 convs + BN + pool + linear).

Math reformulation (exact):
  cin_diff(x_r, x_l) followed by 5x5/stride-5 conv == W1s @ x_l - conv5x5_SAME_pad2(x_r, w1)
  where W1s[o,i] = sum_{a,b} w1[o,i,a,b].

Sharding: pure data-parallel, batch 64 -> 8 cores x 8 images. Conv params
replicated. BN batch stats: each core emits per-channel partial sum / sumsq and
the per-image spatial pool of the conv3 output; the 2KB/core stats reduction and
the final BN-affine + [64,256]@[256,1] linear fold into the host-side unshard
(a device AllGather is available with CIND_TAIL=cc, but on this axon/PJRT setup
cross-core dispatch skew makes the collective cost ~30us of a ~100us kernel).

Layout: channels (256 = 2 chunks of 128) on SBUF partitions; convs are
accumulated PE matmuls over (ci_chunk, tap) with strided access patterns (no
im2col materialization). fp32 path uses float32r (relaxed single-pass matmul);
bf16 path halves weight DMA.
"""

import os
import sys

import numpy as np

if "/opt/trn_rl_repo" not in sys.path:
    sys.path.insert(0, "/opt/trn_rl_repo")

B, C, H, W = 64, 256, 7, 7
NCORES = 8
BPC = B // NCORES  # 8 images per core
BN_EPS = 1e-5

MM_MODE = os.environ.get("CIND_MM_MODE", "bf16")   # bf16 | f32r | f32
TAIL = os.environ.get("CIND_TAIL", "host")          # host | cc
IMPL = os.environ.get("CIND_IMPL", "raw")           # raw | tile
TRACE = False

_CACHE = {}
LAST_RESULT = None


def _build(mode, tail):
    import concourse.bass as bass
    import concourse.tile as tile
    from concourse import mybir

    f32 = mybir.dt.float32
    if mode == "bf16":
        wdt = adt = mybir.dt.bfloat16
    elif mode == "f32":
        wdt = adt = f32
    else:
        # float32r: fp32 storage, relaxed-precision single-pass matmul.
        # The whole conv datapath must be declared f32r (verifier rule).
        wdt = adt = mybir.dt.float32r

    AF = mybir.ActivationFunctionType
    ALU = mybir.AluOpType

    nc = bass.Bass(num_devices=NCORES)

    # ---- per-core DRAM parameters ----
    xr = nc.declare_dram_parameter("xr", [2, 128, BPC, 11, 11], adt, isOutput=False)
    xl = nc.declare_dram_parameter("xl", [2, 128, BPC, 7, 7], adt, isOutput=False)
    w1t = nc.declare_dram_parameter("w1t", [2, 2, 128, 25, 128], wdt, isOutput=False)
    w1s = nc.declare_dram_parameter("w1s", [2, 128, 2, 128], wdt, isOutput=False)
    w2t = nc.declare_dram_parameter("w2t", [2, 2, 128, 9, 128], wdt, isOutput=False)
    w3t = nc.declare_dram_parameter("w3t", [2, 2, 128, 9, 128], wdt, isOutput=False)
    # scal cols: 0:2 b1 | 2:4 b2 | 4:6 b3 | 6:8 gamma | 8:10 beta | 10:12 wl | 12 bl | 13 eps
    scal = nc.declare_dram_parameter("scal", [128, 14], f32, isOutput=False)
    if tail == "cc":
        out_p = nc.declare_dram_parameter("out", [BPC, 1], f32, isOutput=True)
    else:
        pout_p = nc.declare_dram_parameter("pout", [128, 2 * BPC + 4], f32, isOutput=True)

    with tile.TileContext(nc) as tc:
        with (
            tc.tile_pool(name="sb", bufs=1) as sb,
            tc.tile_pool(name="ps", bufs=1, space="PSUM") as ps,
            tc.tile_pool(name="dram", bufs=1, space="DRAM") as dram,
        ):
            # ---- SBUF tiles ----
            scal_t = sb.tile([128, 14], f32, tag="scal", name="scal")
            w1s_t = [sb.tile([128, 2, 128], wdt, tag=f"w1s{i}", name=f"w1s{i}") for i in range(2)]
            xr_t = [sb.tile([128, BPC, 11, 11], adt, tag=f"xr{i}", name=f"xr{i}") for i in range(2)]
            xl_t = [sb.tile([128, BPC, 7, 7], adt, tag=f"xl{i}", name=f"xl{i}") for i in range(2)]
            w1_t = [[sb.tile([128, 25, 128], wdt, tag=f"w1_{i}{o}", name=f"w1_{i}{o}") for o in range(2)]
                    for i in range(2)]
            w2_t = [[sb.tile([128, 9, 128], wdt, tag=f"w2_{i}{o}", name=f"w2_{i}{o}") for o in range(2)]
                    for i in range(2)]
            w3_t = [[sb.tile([128, 9, 128], wdt, tag=f"w3_{i}{o}", name=f"w3_{i}{o}") for o in range(2)]
                    for i in range(2)]

            # small tensors first so the first matmuls can start ASAP, then
            # weights in consumption order, w1 chunks split for earlier start
            nc.sync.dma_start(out=scal_t[:], in_=scal[:])
            # ACT observes scal's DMA lane early so relu biases add no wait
            scr0 = sb.tile([128, 1], f32, tag="scr0", name="scr0")
            nc.scalar.activation(scr0[:], scal_t[:, 12:13], AF.Copy)
            for i in range(2):
                nc.sync.dma_start(out=xl_t[i][:], in_=xl[i])
                nc.sync.dma_start(out=w1s_t[i][:], in_=w1s[i])
            nc.sync.dma_start(out=xr_t[0][:], in_=xr[0])
            for h in range(2):
                sl = slice(0, 13) if h == 0 else slice(13, 25)
                nc.sync.dma_start(out=w1_t[0][0][:, sl, :], in_=w1t[0, 0, :, sl, :])
            nc.sync.dma_start(out=xr_t[1][:], in_=xr[1])
            for i, o in ((1, 0), (0, 1), (1, 1)):
                for h in range(2):
                    sl = slice(0, 13) if h == 0 else slice(13, 25)
                    nc.sync.dma_start(out=w1_t[i][o][:, sl, :], in_=w1t[i, o, :, sl, :])
            for o in range(2):
                for i in range(2):
                    nc.sync.dma_start(out=w2_t[i][o][:], in_=w2t[i, o])
            for o in range(2):
                for i in range(2):
                    nc.sync.dma_start(out=w3_t[i][o][:], in_=w3t[i, o])

            # ---- PE warm-up: keep TensorE busy while w1/xr stream in, so
            # HAM reaches K=8/8 before the real matmuls (and the conv window
            # starts warm). Reads only w1s_t (first small DMA); ~40 N=64 MMs.
            psum_w = ps.tile([128, 64], f32, tag="psum_w", name="psum_w")
            for wi in range(40):
                nc.tensor.matmul(psum_w[:], w1s_t[0][:, 0, :],
                                 w1s_t[0][:, 0, 0:64], start=True, stop=True)

            # ---- conv1: y1 = relu(b1 + W1s@xl - conv5x5_same(xr, w1)) ----
            # (w1t holds -w1, w1s holds +sum(w1); both accumulate into PSUM)
            r1 = [sb.tile([128, BPC, 7, 7], adt, tag=f"r1_{o}", name=f"r1_{o}") for o in range(2)]
            for o in range(2):
                psum1 = ps.tile([128, BPC * 49], f32, tag=f"psum1_{o}", name=f"psum1_{o}")
                n_mm = 52
                k = 0
                for i in range(2):
                    nc.tensor.matmul(
                        psum1[:],
                        w1s_t[i][:, o, :],
                        xl_t[i][:],
                        start=(k == 0), stop=(k == n_mm - 1),
                    )
                    k += 1
                for i in range(2):
                    for a in range(5):
                        for b in range(5):
                            nc.tensor.matmul(
                                psum1[:],
                                w1_t[i][o][:, a * 5 + b, :],
                                xr_t[i][:, :, a:a + 7, b:b + 7],
                                start=(k == 0), stop=(k == n_mm - 1),
                            )
                            k += 1
                nc.scalar.activation(r1[o][:], psum1[:], AF.Relu,
                                     bias=scal_t[:, 0 + o:1 + o])

            # ---- conv2: 3x3 VALID, 7x7 -> 5x5 ----
            r2 = [sb.tile([128, BPC, 5, 5], adt, tag=f"r2_{o}", name=f"r2_{o}") for o in range(2)]
            for o in range(2):
                psum2 = ps.tile([128, BPC * 25], f32, tag=f"psum2_{o}", name=f"psum2_{o}")
                n_mm = 18
                k = 0
                for i in range(2):
                    for a in range(3):
                        for b in range(3):
                            nc.tensor.matmul(
                                psum2[:],
                                w2_t[i][o][:, a * 3 + b, :],
                                r1[i][:, :, a:a + 5, b:b + 5],
                                start=(k == 0), stop=(k == n_mm - 1),
                            )
                            k += 1
                nc.scalar.activation(r2[o][:], psum2[:], AF.Relu,
                                     bias=scal_t[:, 2 + o:3 + o])

            # ---- conv3: 3x3 VALID, 5x5 -> 3x3, + stats ----
            y3 = [sb.tile([128, BPC, 9], f32, tag=f"y3_{o}", name=f"y3_{o}") for o in range(2)]
            sq_scr = sb.tile([128, BPC, 9], f32, tag="sq_scr", name="sq_scr")
            # packed tail output: cols 0:8 ybar0 | 8:16 ybar1 | 16:20 partials
            outsb = sb.tile([128, 2 * BPC + 4], f32, tag="outsb", name="outsb")
            partials = outsb[:, 2 * BPC:]
            ybar = [outsb[:, o * BPC:(o + 1) * BPC] for o in range(2)]
            for o in range(2):
                psum3 = ps.tile([128, BPC * 9], f32, tag=f"psum3_{o}", name=f"psum3_{o}")
                n_mm = 18
                k = 0
                for i in range(2):
                    for a in range(3):
                        for b in range(3):
                            nc.tensor.matmul(
                                psum3[:],
                                w3_t[i][o][:, a * 3 + b, :],
                                r2[i][:, :, a:a + 3, b:b + 3],
                                start=(k == 0), stop=(k == n_mm - 1),
                            )
                            k += 1
                # relu + per-channel sum (accum_out) in one ACT pass
                nc.scalar.activation(y3[o][:], psum3[:], AF.Relu,
                                     bias=scal_t[:, 4 + o:5 + o],
                                     accum_out=partials[:, o:o + 1])
                # sum of squares
                nc.scalar.activation(sq_scr[:], y3[o][:], AF.Square,
                                     accum_out=partials[:, 2 + o:3 + o])
                # per-image spatial sum (AdaptiveAvgPool numerator)
                nc.vector.tensor_reduce(ybar[o], y3[o][:],
                                        axis=mybir.AxisListType.X, op=ALU.add)

            if tail == "host":
                nc.gpsimd.dma_start(out=pout_p[:], in_=outsb[:])
            else:
                # ---- cross-core AllGather of partial stats ----
                cc_in = dram.tile([128, 4], f32, tag="cc_in", name="cc_in")
                cc_out = dram.tile([128 * NCORES, 4], f32, tag="cc_out",
                                   addr_space="Shared", name="cc_out")
                nc.gpsimd.dma_start(out=cc_in[:], in_=partials)
                nc.gpsimd.collective_compute(
                    "AllGather",
                    ALU.bypass,
                    ins=[cc_in[:]],
                    outs=[cc_out[:]],
                    replica_groups=[list(range(NCORES))],
                )
                # gather back: allp[p, c, r] = cc_out[128*r + p, c]
                allp = sb.tile([128, 4, NCORES], f32, tag="allp", name="allp")
                nc.gpsimd.dma_start(
                    out=allp[:],
                    in_=cc_out[:].rearrange("(r p) c -> p c r", r=NCORES),
                )

                # ---- BN scalars ----
                tot = sb.tile([128, 4], f32, tag="tot", name="tot")   # S0 S1 Q0 Q1
                mq = sb.tile([128, 4], f32, tag="mq", name="mq")      # m0 m1 q0 q1
                var = sb.tile([128, 2], f32, tag="var", name="var")
                sd = sb.tile([128, 2], f32, tag="sd", name="sd")
                rstd = sb.tile([128, 2], f32, tag="rstd", name="rstd")
                avec = sb.tile([128, 2], f32, tag="avec", name="avec")
                cbeta = sb.tile([128, 2], f32, tag="cbeta", name="cbeta")
                ones = sb.tile([128, BPC], f32, tag="ones", name="ones")
                nc.vector.memset(ones[:], 1.0)

                nc.vector.tensor_reduce(tot[:], allp[:], axis=mybir.AxisListType.X,
                                        op=ALU.add)
                nc.vector.tensor_scalar_mul(mq[:], tot[:], 1.0 / (B * 9))
                nc.vector.tensor_mul(var[:], mq[:, 0:2], mq[:, 0:2])   # m^2
                nc.vector.tensor_sub(var[:], mq[:, 2:4], var[:])       # q - m^2
                nc.scalar.activation(sd[:], var[:], AF.Sqrt, bias=scal_t[:, 13:14])
                nc.vector.reciprocal(rstd[:], sd[:])
                # A0 = wl * gamma * rstd ; const_c = wl*beta - A0*mean ; A = A0/9
                cmean = sb.tile([128, 2], f32, tag="cmean", name="cmean")
                nc.vector.tensor_mul(avec[:], rstd[:], scal_t[:, 6:8])
                nc.vector.tensor_mul(avec[:], avec[:], scal_t[:, 10:12])
                nc.vector.tensor_mul(cmean[:], avec[:], mq[:, 0:2])
                nc.vector.tensor_mul(cbeta[:], scal_t[:, 8:10], scal_t[:, 10:12])
                nc.vector.tensor_sub(cbeta[:], cbeta[:], cmean[:])
                nc.vector.tensor_scalar_mul(avec[:], avec[:], 1.0 / 9)

                # ---- out_b = sum_c A_c ybar_bc + sum_c Cb_c + bl ----
                psum_o = ps.tile([1, BPC], f32, tag="psum_o", name="psum_o")
                for o in range(2):
                    nc.tensor.matmul(psum_o[:], avec[:, o:o + 1], ybar[o],
                                     start=(o == 0), stop=False)
                for o in range(2):
                    nc.tensor.matmul(psum_o[:], cbeta[:, o:o + 1], ones[:],
                                     start=False, stop=(o == 1))
                outv = sb.tile([1, BPC], f32, tag="outv", name="outv")
                nc.scalar.activation(outv[:], psum_o[:], AF.Identity,
                                     bias=scal_t[0:1, 12:13])
                nc.gpsimd.dma_start(out=out_p[:], in_=outv[:])

    _split_multiwaits(nc, mybir)
    nc.finalize()
    return nc


def _split_multiwaits(nc, mybir):
    """walrus codegen allows at most ONE sync-wait per instruction. Tile's
    joins (and its kernel-tail drain) can carry several; split the extras
    into single-wait NOPs on the same engine immediately before the
    instruction (engines execute serially, so sequential waits == AND)."""
    for fn in nc.m.functions:
        for bb in fn.blocks:
            new_list = []
            for inst in bb.instructions:
                si = inst.sync_info
                if si is not None and si.on_wait and len(si.on_wait) > 1:
                    waits = list(si.on_wait)
                    for j, w in enumerate(waits[:-1]):
                        nop = mybir.InstNoOp(
                            name=f"{inst.name}_w{j}",
                            sync_info=mybir.SyncInfo(on_wait=[w], on_update=[]),
                            engine=inst.engine,
                            bass_nofuse=True,
                        )
                        nc.register_instruction(nop)
                        new_list.append(nop)
                    si.on_wait = [waits[-1]]
                new_list.append(inst)
            bb.instructions[:] = new_list


def _build_raw(mode):
    """Raw-Block implementation (bf16 + host tail only): hand-placed
    semaphores instead of TileContext, eliminating Tile's ~14us of entry
    barriers + exit drain/butterfly. Dependency structure:
      sync: all input DMAs in consumption order, each +16 on dma_sem
      PE:   warm-up MMs, then conv groups gated on dma_sem/act_sem ticks
      ACT:  relu(+bias) per psum group gated on pe_sem; stats accumulators
      DVE:  per-image pooling gated on act_sem
      gpsimd: packed output DMA after everything, then sem cleanup
    """
    import concourse.bass as bass
    from concourse import mybir

    assert mode == "bf16"
    f32 = mybir.dt.float32
    dt = mybir.dt.bfloat16
    AF = mybir.ActivationFunctionType
    ALU = mybir.AluOpType

    nc = bass.Bass(num_devices=NCORES)

    xr = nc.declare_dram_parameter("xr", [2, 128, BPC, 11, 11], dt, isOutput=False)
    xl = nc.declare_dram_parameter("xl", [2, 128, BPC, 7, 7], dt, isOutput=False)
    w1t = nc.declare_dram_parameter("w1t", [2, 2, 128, 25, 128], dt, isOutput=False)
    w1s = nc.declare_dram_parameter("w1s", [2, 128, 2, 128], dt, isOutput=False)
    w2t = nc.declare_dram_parameter("w2t", [2, 2, 128, 9, 128], dt, isOutput=False)
    w3t = nc.declare_dram_parameter("w3t", [2, 2, 128, 9, 128], dt, isOutput=False)
    scal = nc.declare_dram_parameter("scal", [128, 14], f32, isOutput=False)
    pout_p = nc.declare_dram_parameter("pout", [128, 2 * BPC + 4], f32, isOutput=True)

    from contextlib import ExitStack
    NLANES = 8
    with ExitStack() as ctx:
        dma_sems = [ctx.enter_context(nc.semaphore(f"dma{j}")) for j in range(NLANES)]
        out_sem = ctx.enter_context(nc.semaphore("out_sem"))
        pe_sem = ctx.enter_context(nc.semaphore("pe_sem"))
        act_sem = ctx.enter_context(nc.semaphore("act_sem"))
        dve_sem = ctx.enter_context(nc.semaphore("dve_sem"))

        def sb(name, shape, d):
            return ctx.enter_context(nc.sbuf_tensor(name, shape, d))

        def pst(name, shape):
            return ctx.enter_context(nc.psum_tensor(name, shape, f32))

        scal_t = sb("scal_t", [128, 14], f32)
        scr0 = sb("scr0", [128, 1], f32)
        xl0, xl1 = sb("xl0", [128, BPC, 7, 7], dt), sb("xl1", [128, BPC, 7, 7], dt)
        w1s0, w1s1 = sb("w1s0", [128, 2, 128], dt), sb("w1s1", [128, 2, 128], dt)
        xr0, xr1 = sb("xr0", [128, BPC, 11, 11], dt), sb("xr1", [128, BPC, 11, 11], dt)
        w1_00, w1_10 = sb("w1_00", [128, 25, 128], dt), sb("w1_10", [128, 25, 128], dt)
        w1_01, w1_11 = sb("w1_01", [128, 25, 128], dt), sb("w1_11", [128, 25, 128], dt)
        w2_00, w2_10 = sb("w2_00", [128, 9, 128], dt), sb("w2_10", [128, 9, 128], dt)
        w2_01, w2_11 = sb("w2_01", [128, 9, 128], dt), sb("w2_11", [128, 9, 128], dt)
        w3_00, w3_10 = sb("w3_00", [128, 9, 128], dt), sb("w3_10", [128, 9, 128], dt)
        w3_01, w3_11 = sb("w3_01", [128, 9, 128], dt), sb("w3_11", [128, 9, 128], dt)
        r1_0, r1_1 = sb("r1_0", [128, BPC, 7, 7], dt), sb("r1_1", [128, BPC, 7, 7], dt)
        r2_0, r2_1 = sb("r2_0", [128, BPC, 5, 5], dt), sb("r2_1", [128, BPC, 5, 5], dt)
        y3_0, y3_1 = sb("y3_0", [128, BPC, 9], f32), sb("y3_1", [128, BPC, 9], f32)
        sq_scr = sb("sq_scr", [128, BPC, 9], f32)
        outsb = sb("outsb", [128, 2 * BPC + 4], f32)
        # one full 2KB bank per psum tensor (group + bank-safety isolation)
        psum_w = pst("psum_w", [128, 512])[:, 0:64]
        psum1_0 = pst("psum1_0", [128, 512])[:, 0:BPC * 49]
        psum1_1 = pst("psum1_1", [128, 512])[:, 0:BPC * 49]
        psum2_0 = pst("psum2_0", [128, 512])[:, 0:BPC * 25]
        psum2_1 = pst("psum2_1", [128, 512])[:, 0:BPC * 25]
        psum3_0 = pst("psum3_0", [128, 512])[:, 0:BPC * 9]
        psum3_1 = pst("psum3_1", [128, 512])[:, 0:BPC * 9]

        w1sb = [w1s0, w1s1]
        xlb = [xl0, xl1]
        xrb = [xr0, xr1]
        w1b = [[w1_00, w1_01], [w1_10, w1_11]]
        w2b = [[w2_00, w2_01], [w2_10, w2_11]]
        w3b = [[w3_00, w3_01], [w3_10, w3_11]]
        r1b, r2b, y3b = [r1_0, r1_1], [r2_0, r2_1], [y3_0, y3_1]
        psum1 = [psum1_0, psum1_1]
        psum2 = [psum2_0, psum2_1]
        psum3 = [psum3_0, psum3_1]
        partials = outsb[:, 2 * BPC:]
        ybar = [outsb[:, o * BPC:(o + 1) * BPC] for o in range(2)]

        # DMA completion ticks (1-based, each DMA adds 16)
        D = {}

        with nc.Block() as block:

            @block.sync
            def _(sync):
                k = 0
                lane_cnt = [0] * NLANES

                def dma(name, out, in_):
                    nonlocal k
                    lane = k % NLANES
                    # serialize same-lane DMAs through completion so the
                    # lane sem value is unambiguous (Tile's protocol)
                    if lane_cnt[lane] > 0:
                        sync.wait_ge(dma_sems[lane], 16 * lane_cnt[lane])
                    sync.dma_start(out=out, in_=in_).then_inc(dma_sems[lane], 16)
                    lane_cnt[lane] += 1
                    k += 1
                    D[name] = (lane, 16 * lane_cnt[lane])

                dma("scal", scal_t[:], scal[:])
                dma("xl0", xl0[:], xl[0])
                dma("w1s0", w1s0[:], w1s[0])
                dma("xl1", xl1[:], xl[1])
                dma("w1s1", w1s1[:], w1s[1])
                dma("xr0", xr0[:], xr[0])
                dma("w1_00a", w1_00[:, 0:13, :], w1t[0, 0, :, 0:13, :])
                dma("w1_00b", w1_00[:, 13:25, :], w1t[0, 0, :, 13:25, :])
                dma("xr1", xr1[:], xr[1])
                dma("w1_10a", w1_10[:, 0:13, :], w1t[1, 0, :, 0:13, :])
                dma("w1_10b", w1_10[:, 13:25, :], w1t[1, 0, :, 13:25, :])
                dma("w1_01a", w1_01[:, 0:13, :], w1t[0, 1, :, 0:13, :])
                dma("w1_01b", w1_01[:, 13:25, :], w1t[0, 1, :, 13:25, :])
                dma("w1_11a", w1_11[:, 0:13, :], w1t[1, 1, :, 0:13, :])
                dma("w1_11b", w1_11[:, 13:25, :], w1t[1, 1, :, 13:25, :])
                dma("w2_00", w2_00[:], w2t[0, 0])
                dma("w2_10", w2_10[:], w2t[1, 0])
                dma("w2_01", w2_01[:], w2t[0, 1])
                dma("w2_11", w2_11[:], w2t[1, 1])
                dma("w3_00", w3_00[:], w3t[0, 0])
                dma("w3_10", w3_10[:], w3t[1, 0])
                dma("w3_01", w3_01[:], w3t[0, 1])
                dma("w3_11", w3_11[:], w3t[1, 1])

            @block.tensor
            def _(pe):
                # warm-up while weights stream in (HAM to K=8/8)
                pe.wait_ge(dma_sems[D["w1s0"][0]], D["w1s0"][1])
                for _i in range(40):
                    pe.matmul(psum_w, w1s0[:, 0, :], w1s0[:, 0, 0:64],
                              start=True, stop=True)

                # conv1 (52 accumulating MMs per output chunk)
                for o in range(2):
                    first = (o == 0)
                    for nm in ("xl0", "w1s0", "xl1", "w1s1"):
                        pe.wait_ge(dma_sems[D[nm][0]], D[nm][1])
                    for i in range(2):
                        pe.matmul(psum1[o], w1sb[i][:, o, :], xlb[i][:],
                                  start=(i == 0), stop=False)
                    for i in range(2):
                        for h, taps in ((0, range(0, 13)), (1, range(13, 25))):
                            for nm in ((f"xr{i}", f"w1_{i}{o}a") if h == 0
                                       else (f"w1_{i}{o}b",)):
                                pe.wait_ge(dma_sems[D[nm][0]], D[nm][1])
                            for t in taps:
                                a, b = divmod(t, 5)
                                last = (i == 1 and t == 24)
                                mm = pe.matmul(psum1[o], w1b[i][o][:, t, :],
                                               xrb[i][:, :, a:a + 7, b:b + 7],
                                               start=False, stop=last)
                                if last:
                                    mm.then_inc(pe_sem, 1)

                # conv2
                for o in range(2):
                    pe.wait_ge(act_sem, 3)   # scr0 + r1_0 + r1_1
                    for nm in (f"w2_0{o}", f"w2_1{o}"):
                        pe.wait_ge(dma_sems[D[nm][0]], D[nm][1])
                    k = 0
                    for i in range(2):
                        for t in range(9):
                            a, b = divmod(t, 3)
                            mm = pe.matmul(psum2[o], w2b[i][o][:, t, :],
                                           r1b[i][:, :, a:a + 5, b:b + 5],
                                           start=(k == 0), stop=(k == 17))
                            if k == 17:
                                mm.then_inc(pe_sem, 1)
                            k += 1

                # conv3
                for o in range(2):
                    pe.wait_ge(act_sem, 5)   # + r2_0 + r2_1
                    for nm in (f"w3_0{o}", f"w3_1{o}"):
                        pe.wait_ge(dma_sems[D[nm][0]], D[nm][1])
                    k = 0
                    for i in range(2):
                        for t in range(9):
                            a, b = divmod(t, 3)
                            mm = pe.matmul(psum3[o], w3b[i][o][:, t, :],
                                           r2b[i][:, :, a:a + 3, b:b + 3],
                                           start=(k == 0), stop=(k == 17))
                            if k == 17:
                                mm.then_inc(pe_sem, 1)
                            k += 1

            @block.scalar
            def _(act):
                # touch scal early: preloads ACT table during the DMA window
                act.wait_ge(dma_sems[D["scal"][0]], D["scal"][1])
                act.activation(scr0[:], scal_t[:, 12:13], AF.Copy).then_inc(
                    act_sem, 1)
                for o in range(2):           # r1 = relu(psum1 + b1)
                    act.wait_ge(pe_sem, 1 + o)
                    act.activation(r1b[o][:], psum1[o], AF.Relu,
                                   bias=scal_t[:, 0 + o:1 + o]).then_inc(act_sem, 1)
                for o in range(2):           # r2 = relu(psum2 + b2)
                    act.wait_ge(pe_sem, 3 + o)
                    act.activation(r2b[o][:], psum2[o], AF.Relu,
                                   bias=scal_t[:, 2 + o:3 + o]).then_inc(act_sem, 1)
                for o in range(2):           # y3 = relu(psum3 + b3) + stats
                    act.wait_ge(pe_sem, 5 + o)
                    act.activation(y3b[o][:], psum3[o], AF.Relu,
                                   bias=scal_t[:, 4 + o:5 + o],
                                   accum_out=partials[:, o:o + 1]).then_inc(
                        act_sem, 1)
                    # self-wait: ACT pipelines, so the Square reading y3 must
                    # wait for the relu's completion tick
                    act.wait_ge(act_sem, 6 + 2 * o)
                    act.activation(sq_scr[:], y3b[o][:], AF.Square,
                                   accum_out=partials[:, 2 + o:3 + o]).then_inc(
                        act_sem, 1)

            @block.vector
            def _(dve):
                for o in range(2):           # ybar = per-image spatial sum
                    dve.wait_ge(act_sem, 6 + 2 * o)   # y3[o] relu done
                    dve.tensor_reduce(ybar[o], y3b[o][:],
                                      axis=mybir.AxisListType.X,
                                      op=ALU.add).then_inc(dve_sem, 1)

            @block.gpsimd
            def _(gp):
                gp.wait_ge(act_sem, 9)
                gp.wait_ge(dve_sem, 2)
                gp.dma_start(out=pout_p[:], in_=outsb[:]).then_inc(out_sem, 16)
                gp.wait_ge(out_sem, 16)
                # (no sem_clear: NRT re-initializes semaphores per execution;
                # verified by the repeated-run correctness check in test.py)

    _split_multiwaits(nc, mybir)
    nc.finalize()
    return nc


def _np_dt(mode):
    if mode == "bf16":
        import ml_dtypes
        return ml_dtypes.bfloat16
    return np.float32


def _prep_inputs(inputs, mode):
    adt = _np_dt(mode)
    wdt = _np_dt(mode)

    x_r = np.asarray(inputs["x_r"], np.float32)
    x_l = np.asarray(inputs["x_l"], np.float32)
    w1 = np.asarray(inputs["w1"], np.float32)
    w2 = np.asarray(inputs["w2"], np.float32)
    w3 = np.asarray(inputs["w3"], np.float32)

    xp = np.pad(x_r, ((0, 0), (0, 0), (2, 2), (2, 2)))

    # lhsT layouts: [ci_chunk, co_chunk, ci_p, tap, co_p]
    w1t = np.ascontiguousarray(
        (-w1).transpose(1, 2, 3, 0).reshape(2, 128, 25, 2, 128)
        .transpose(0, 3, 1, 2, 4).astype(wdt))
    w1sum = np.ascontiguousarray(
        w1.sum(axis=(2, 3)).transpose(1, 0).reshape(2, 128, 2, 128).astype(wdt))
    w2t = np.ascontiguousarray(
        w2.transpose(1, 2, 3, 0).reshape(2, 128, 9, 2, 128)
        .transpose(0, 3, 1, 2, 4).astype(wdt))
    w3t = np.ascontiguousarray(
        w3.transpose(1, 2, 3, 0).reshape(2, 128, 9, 2, 128)
        .transpose(0, 3, 1, 2, 4).astype(wdt))

    scal = np.zeros((128, 14), np.float32)
    for col, name in ((0, "b1"), (2, "b2"), (4, "b3"), (6, "gamma"), (8, "beta")):
        scal[:, col:col + 2] = np.asarray(inputs[name], np.float32).reshape(2, 128).T
    scal[:, 10:12] = np.asarray(inputs["wl"], np.float32).reshape(2, 128).T
    scal[:, 12] = np.asarray(inputs["bl"], np.float32)[0]
    scal[:, 13] = BN_EPS

    in_maps = []
    for k in range(NCORES):
        sl = slice(k * BPC, (k + 1) * BPC)
        xr_k = np.ascontiguousarray(
            xp[sl].transpose(1, 0, 2, 3).reshape(2, 128, BPC, 11, 11).astype(adt))
        xl_k = np.ascontiguousarray(
            x_l[sl].transpose(1, 0, 2, 3).reshape(2, 128, BPC, 7, 7).astype(adt))
        in_maps.append({
            "xr": xr_k, "xl": xl_k,
            "w1t": w1t, "w1s": w1sum, "w2t": w2t, "w3t": w3t,
            "scal": scal,
        })
    return in_maps


def kernel(**inputs):
    global LAST_RESULT
    from concourse.bass_utils import run_bass_kernel_spmd

    mode, tail, impl = MM_MODE, TAIL, IMPL
    if impl == "raw" and (mode != "bf16" or tail != "host"):
        impl = "tile"
    key = (mode, tail, impl)
    if key not in _CACHE:
        _CACHE[key] = _build_raw(mode) if impl == "raw" else _build(mode, tail)
    nc = _CACHE[key]

    in_maps = _prep_inputs(inputs, mode)
    res = run_bass_kernel_spmd(nc, in_maps, list(range(NCORES)), trace=TRACE)
    LAST_RESULT = res

    return _postprocess(res.results, inputs, tail)


def _postprocess(results, inputs, tail):
    if tail == "cc":
        out = np.concatenate([r["out"] for r in results], axis=0)
        return out.astype(np.float32)

    # host-side unshard: combine per-core BN partials, apply affine + linear
    packed = np.stack([np.asarray(r["pout"], np.float32) for r in results])  # [8,128,20]
    ybar = np.stack([packed[:, :, 0:BPC], packed[:, :, BPC:2 * BPC]], axis=1)
    ybar = ybar.transpose(0, 1, 2, 3)                          # [8, 2, 128, 8]
    pout = packed[:, :, 2 * BPC:]                              # [8, 128, 4]
    tot = pout.sum(axis=0)                                     # [128, 4]
    n = float(B * 9)
    mean = (tot[:, 0:2] / n).T.reshape(C)                      # channel c = o*128+p
    q = (tot[:, 2:4] / n).T.reshape(C)
    var = q - mean * mean
    rstd = 1.0 / np.sqrt(var + BN_EPS)
    wl = np.asarray(inputs["wl"], np.float32).reshape(C)
    gamma = np.asarray(inputs["gamma"], np.float32).reshape(C)
    beta = np.asarray(inputs["beta"], np.float32).reshape(C)
    bl = np.asarray(inputs["bl"], np.float32).reshape(1)
    a0 = wl * gamma * rstd
    const = bl[0] + np.sum(wl * beta) - np.sum(a0 * mean)
    yb = ybar.transpose(0, 3, 1, 2).reshape(B, C)              # [64, 256] (c=o*128+p)
    out = (yb / 9.0) @ a0 + const
    return out.astype(np.float32).reshape(B, 1)


# revision 27
# speedup vs baseline: 1.0340x; 1.0340x over previous
"""Trainium2 Bass kernel for nn_CIND_Block (cin_diff + 3 convs + BN + pool + linear).

Math reformulation (exact):
  cin_diff(x_r, x_l) followed by 5x5/stride-5 conv == W1s @ x_l - conv5x5_SAME_pad2(x_r, w1)
  where W1s[o,i] = sum_{a,b} w1[o,i,a,b].

Sharding: pure data-parallel, batch 64 -> 8 cores x 8 images. Conv params
replicated. BN batch stats: each core emits per-channel partial sum / sumsq and
the per-image spatial pool of the conv3 output; the 2KB/core stats reduction and
the final BN-affine + [64,256]@[256,1] linear fold into the host-side unshard
(a device AllGather is available with CIND_TAIL=cc, but on this axon/PJRT setup
cross-core dispatch skew makes the collective cost ~30us of a ~100us kernel).

Layout: channels (256 = 2 chunks of 128) on SBUF partitions; convs are
accumulated PE matmuls over (ci_chunk, tap) with strided access patterns (no
im2col materialization). fp32 path uses float32r (relaxed single-pass matmul);
bf16 path halves weight DMA.
"""

import os
import sys

import numpy as np

if "/opt/trn_rl_repo" not in sys.path:
    sys.path.insert(0, "/opt/trn_rl_repo")

B, C, H, W = 64, 256, 7, 7
NCORES = 8
BPC = B // NCORES  # 8 images per core
BN_EPS = 1e-5

MM_MODE = os.environ.get("CIND_MM_MODE", "bf16")   # bf16 | f32r | f32
TAIL = os.environ.get("CIND_TAIL", "host")          # host | cc
IMPL = os.environ.get("CIND_IMPL", "raw")           # raw | tile
TRACE = False

_CACHE = {}
LAST_RESULT = None


def _build(mode, tail):
    import concourse.bass as bass
    import concourse.tile as tile
    from concourse import mybir

    f32 = mybir.dt.float32
    if mode == "bf16":
        wdt = adt = mybir.dt.bfloat16
    elif mode == "f32":
        wdt = adt = f32
    else:
        # float32r: fp32 storage, relaxed-precision single-pass matmul.
        # The whole conv datapath must be declared f32r (verifier rule).
        wdt = adt = mybir.dt.float32r

    AF = mybir.ActivationFunctionType
    ALU = mybir.AluOpType

    nc = bass.Bass(num_devices=NCORES)

    # ---- per-core DRAM parameters ----
    xr = nc.declare_dram_parameter("xr", [2, 128, BPC, 11, 11], adt, isOutput=False)
    xl = nc.declare_dram_parameter("xl", [2, 128, BPC, 7, 7], adt, isOutput=False)
    w1t = nc.declare_dram_parameter("w1t", [2, 2, 128, 25, 128], wdt, isOutput=False)
    w1s = nc.declare_dram_parameter("w1s", [2, 128, 2, 128], wdt, isOutput=False)
    w2t = nc.declare_dram_parameter("w2t", [2, 2, 128, 9, 128], wdt, isOutput=False)
    w3t = nc.declare_dram_parameter("w3t", [2, 2, 128, 9, 128], wdt, isOutput=False)
    # scal cols: 0:2 b1 | 2:4 b2 | 4:6 b3 | 6:8 gamma | 8:10 beta | 10:12 wl | 12 bl | 13 eps
    scal = nc.declare_dram_parameter("scal", [128, 14], f32, isOutput=False)
    if tail == "cc":
        out_p = nc.declare_dram_parameter("out", [BPC, 1], f32, isOutput=True)
    else:
        pout_p = nc.declare_dram_parameter("pout", [128, 2 * BPC + 4], f32, isOutput=True)

    with tile.TileContext(nc) as tc:
        with (
            tc.tile_pool(name="sb", bufs=1) as sb,
            tc.tile_pool(name="ps", bufs=1, space="PSUM") as ps,
            tc.tile_pool(name="dram", bufs=1, space="DRAM") as dram,
        ):
            # ---- SBUF tiles ----
            scal_t = sb.tile([128, 14], f32, tag="scal", name="scal")
            w1s_t = [sb.tile([128, 2, 128], wdt, tag=f"w1s{i}", name=f"w1s{i}") for i in range(2)]
            xr_t = [sb.tile([128, BPC, 11, 11], adt, tag=f"xr{i}", name=f"xr{i}") for i in range(2)]
            xl_t = [sb.tile([128, BPC, 7, 7], adt, tag=f"xl{i}", name=f"xl{i}") for i in range(2)]
            w1_t = [[sb.tile([128, 25, 128], wdt, tag=f"w1_{i}{o}", name=f"w1_{i}{o}") for o in range(2)]
                    for i in range(2)]
            w2_t = [[sb.tile([128, 9, 128], wdt, tag=f"w2_{i}{o}", name=f"w2_{i}{o}") for o in range(2)]
                    for i in range(2)]
            w3_t = [[sb.tile([128, 9, 128], wdt, tag=f"w3_{i}{o}", name=f"w3_{i}{o}") for o in range(2)]
                    for i in range(2)]

            # small tensors first so the first matmuls can start ASAP, then
            # weights in consumption order, w1 chunks split for earlier start
            nc.sync.dma_start(out=scal_t[:], in_=scal[:])
            # ACT observes scal's DMA lane early so relu biases add no wait
            scr0 = sb.tile([128, 1], f32, tag="scr0", name="scr0")
            nc.scalar.activation(scr0[:], scal_t[:, 12:13], AF.Copy)
            for i in range(2):
                nc.sync.dma_start(out=xl_t[i][:], in_=xl[i])
                nc.sync.dma_start(out=w1s_t[i][:], in_=w1s[i])
            nc.sync.dma_start(out=xr_t[0][:], in_=xr[0])
            for h in range(2):
                sl = slice(0, 13) if h == 0 else slice(13, 25)
                nc.sync.dma_start(out=w1_t[0][0][:, sl, :], in_=w1t[0, 0, :, sl, :])
            nc.sync.dma_start(out=xr_t[1][:], in_=xr[1])
            for i, o in ((1, 0), (0, 1), (1, 1)):
                for h in range(2):
                    sl = slice(0, 13) if h == 0 else slice(13, 25)
                    nc.sync.dma_start(out=w1_t[i][o][:, sl, :], in_=w1t[i, o, :, sl, :])
            for o in range(2):
                for i in range(2):
                    nc.sync.dma_start(out=w2_t[i][o][:], in_=w2t[i, o])
            for o in range(2):
                for i in range(2):
                    nc.sync.dma_start(out=w3_t[i][o][:], in_=w3t[i, o])

            # ---- PE warm-up: keep TensorE busy while w1/xr stream in, so
            # HAM reaches K=8/8 before the real matmuls (and the conv window
            # starts warm). Reads only w1s_t (first small DMA); ~40 N=64 MMs.
            psum_w = ps.tile([128, 64], f32, tag="psum_w", name="psum_w")
            for wi in range(40):
                nc.tensor.matmul(psum_w[:], w1s_t[0][:, 0, :],
                                 w1s_t[0][:, 0, 0:64], start=True, stop=True)

            # ---- conv1: y1 = relu(b1 + W1s@xl - conv5x5_same(xr, w1)) ----
            # (w1t holds -w1, w1s holds +sum(w1); both accumulate into PSUM)
            r1 = [sb.tile([128, BPC, 7, 7], adt, tag=f"r1_{o}", name=f"r1_{o}") for o in range(2)]
            for o in range(2):
                psum1 = ps.tile([128, BPC * 49], f32, tag=f"psum1_{o}", name=f"psum1_{o}")
                n_mm = 52
                k = 0
                for i in range(2):
                    nc.tensor.matmul(
                        psum1[:],
                        w1s_t[i][:, o, :],
                        xl_t[i][:],
                        start=(k == 0), stop=(k == n_mm - 1),
                    )
                    k += 1
                for i in range(2):
                    for a in range(5):
                        for b in range(5):
                            nc.tensor.matmul(
                                psum1[:],
                                w1_t[i][o][:, a * 5 + b, :],
                                xr_t[i][:, :, a:a + 7, b:b + 7],
                                start=(k == 0), stop=(k == n_mm - 1),
                            )
                            k += 1
                nc.scalar.activation(r1[o][:], psum1[:], AF.Relu,
                                     bias=scal_t[:, 0 + o:1 + o])

            # ---- conv2: 3x3 VALID, 7x7 -> 5x5 ----
            r2 = [sb.tile([128, BPC, 5, 5], adt, tag=f"r2_{o}", name=f"r2_{o}") for o in range(2)]
            for o in range(2):
                psum2 = ps.tile([128, BPC * 25], f32, tag=f"psum2_{o}", name=f"psum2_{o}")
                n_mm = 18
                k = 0
                for i in range(2):
                    for a in range(3):
                        for b in range(3):
                            nc.tensor.matmul(
                                psum2[:],
                                w2_t[i][o][:, a * 3 + b, :],
                                r1[i][:, :, a:a + 5, b:b + 5],
                                start=(k == 0), stop=(k == n_mm - 1),
                            )
                            k += 1
                nc.scalar.activation(r2[o][:], psum2[:], AF.Relu,
                                     bias=scal_t[:, 2 + o:3 + o])

            # ---- conv3: 3x3 VALID, 5x5 -> 3x3, + stats ----
            y3 = [sb.tile([128, BPC, 9], f32, tag=f"y3_{o}", name=f"y3_{o}") for o in range(2)]
            sq_scr = sb.tile([128, BPC, 9], f32, tag="sq_scr", name="sq_scr")
            # packed tail output: cols 0:8 ybar0 | 8:16 ybar1 | 16:20 partials
            outsb = sb.tile([128, 2 * BPC + 4], f32, tag="outsb", name="outsb")
            partials = outsb[:, 2 * BPC:]
            ybar = [outsb[:, o * BPC:(o + 1) * BPC] for o in range(2)]
            for o in range(2):
                psum3 = ps.tile([128, BPC * 9], f32, tag=f"psum3_{o}", name=f"psum3_{o}")
                n_mm = 18
                k = 0
                for i in range(2):
                    for a in range(3):
                        for b in range(3):
                            nc.tensor.matmul(
                                psum3[:],
                                w3_t[i][o][:, a * 3 + b, :],
                                r2[i][:, :, a:a + 3, b:b + 3],
                                start=(k == 0), stop=(k == n_mm - 1),
                            )
                            k += 1
                # relu + per-channel sum (accum_out) in one ACT pass
                nc.scalar.activation(y3[o][:], psum3[:], AF.Relu,
                                     bias=scal_t[:, 4 + o:5 + o],
                                     accum_out=partials[:, o:o + 1])
                # sum of squares
                nc.scalar.activation(sq_scr[:], y3[o][:], AF.Square,
                                     accum_out=partials[:, 2 + o:3 + o])
                # per-image spatial sum (AdaptiveAvgPool numerator)
                nc.vector.tensor_reduce(ybar[o], y3[o][:],
                                        axis=mybir.AxisListType.X, op=ALU.add)

            if tail == "host":
                nc.gpsimd.dma_start(out=pout_p[:], in_=outsb[:])
            else:
                # ---- cross-core AllGather of partial stats ----
                cc_in = dram.tile([128, 4], f32, tag="cc_in", name="cc_in")
                cc_out = dram.tile([128 * NCORES, 4], f32, tag="cc_out",
                                   addr_space="Shared", name="cc_out")
                nc.gpsimd.dma_start(out=cc_in[:], in_=partials)
                nc.gpsimd.collective_compute(
                    "AllGather",
                    ALU.bypass,
                    ins=[cc_in[:]],
                    outs=[cc_out[:]],
                    replica_groups=[list(range(NCORES))],
                )
                # gather back: allp[p, c, r] = cc_out[128*r + p, c]
                allp = sb.tile([128, 4, NCORES], f32, tag="allp", name="allp")
                nc.gpsimd.dma_start(
                    out=allp[:],
                    in_=cc_out[:].rearrange("(r p) c -> p c r", r=NCORES),
                )

                # ---- BN scalars ----
                tot = sb.tile([128, 4], f32, tag="tot", name="tot")   # S0 S1 Q0 Q1
                mq = sb.tile([128, 4], f32, tag="mq", name="mq")      # m0 m1 q0 q1
                var = sb.tile([128, 2], f32, tag="var", name="var")
                sd = sb.tile([128, 2], f32, tag="sd", name="sd")
                rstd = sb.tile([128, 2], f32, tag="rstd", name="rstd")
                avec = sb.tile([128, 2], f32, tag="avec", name="avec")
                cbeta = sb.tile([128, 2], f32, tag="cbeta", name="cbeta")
                ones = sb.tile([128, BPC], f32, tag="ones", name="ones")
                nc.vector.memset(ones[:], 1.0)

                nc.vector.tensor_reduce(tot[:], allp[:], axis=mybir.AxisListType.X,
                                        op=ALU.add)
                nc.vector.tensor_scalar_mul(mq[:], tot[:], 1.0 / (B * 9))
                nc.vector.tensor_mul(var[:], mq[:, 0:2], mq[:, 0:2])   # m^2
                nc.vector.tensor_sub(var[:], mq[:, 2:4], var[:])       # q - m^2
                nc.scalar.activation(sd[:], var[:], AF.Sqrt, bias=scal_t[:, 13:14])
                nc.vector.reciprocal(rstd[:], sd[:])
                # A0 = wl * gamma * rstd ; const_c = wl*beta - A0*mean ; A = A0/9
                cmean = sb.tile([128, 2], f32, tag="cmean", name="cmean")
                nc.vector.tensor_mul(avec[:], rstd[:], scal_t[:, 6:8])
                nc.vector.tensor_mul(avec[:], avec[:], scal_t[:, 10:12])
                nc.vector.tensor_mul(cmean[:], avec[:], mq[:, 0:2])
                nc.vector.tensor_mul(cbeta[:], scal_t[:, 8:10], scal_t[:, 10:12])
                nc.vector.tensor_sub(cbeta[:], cbeta[:], cmean[:])
                nc.vector.tensor_scalar_mul(avec[:], avec[:], 1.0 / 9)

                # ---- out_b = sum_c A_c ybar_bc + sum_c Cb_c + bl ----
                psum_o = ps.tile([1, BPC], f32, tag="psum_o", name="psum_o")
                for o in range(2):
                    nc.tensor.matmul(psum_o[:], avec[:, o:o + 1], ybar[o],
                                     start=(o == 0), stop=False)
                for o in range(2):
                    nc.tensor.matmul(psum_o[:], cbeta[:, o:o + 1], ones[:],
                                     start=False, stop=(o == 1))
                outv = sb.tile([1, BPC], f32, tag="outv", name="outv")
                nc.scalar.activation(outv[:], psum_o[:], AF.Identity,
                                     bias=scal_t[0:1, 12:13])
                nc.gpsimd.dma_start(out=out_p[:], in_=outv[:])

    _split_multiwaits(nc, mybir)
    nc.finalize()
    return nc


def _split_multiwaits(nc, mybir):
    """walrus codegen allows at most ONE sync-wait per instruction. Tile's
    joins (and its kernel-tail drain) can carry several; split the extras
    into single-wait NOPs on the same engine immediately before the
    instruction (engines execute serially, so sequential waits == AND)."""
    for fn in nc.m.functions:
        for bb in fn.blocks:
            new_list = []
            for inst in bb.instructions:
                si = inst.sync_info
                if si is not None and si.on_wait and len(si.on_wait) > 1:
                    waits = list(si.on_wait)
                    for j, w in enumerate(waits[:-1]):
                        nop = mybir.InstNoOp(
                            name=f"{inst.name}_w{j}",
                            sync_info=mybir.SyncInfo(on_wait=[w], on_update=[]),
                            engine=inst.engine,
                            bass_nofuse=True,
                        )
                        nc.register_instruction(nop)
                        new_list.append(nop)
                    si.on_wait = [waits[-1]]
                new_list.append(inst)
            bb.instructions[:] = new_list


def _build_raw(mode):
    """Raw-Block implementation (bf16 + host tail only): hand-placed
    semaphores instead of TileContext, eliminating Tile's ~14us of entry
    barriers + exit drain/butterfly. Dependency structure:
      sync: all input DMAs in consumption order, each +16 on dma_sem
      PE:   warm-up MMs, then conv groups gated on dma_sem/act_sem ticks
      ACT:  relu(+bias) per psum group gated on pe_sem; stats accumulators
      DVE:  per-image pooling gated on act_sem
      gpsimd: packed output DMA after everything, then sem cleanup
    """
    import concourse.bass as bass
    from concourse import mybir

    assert mode == "bf16"
    f32 = mybir.dt.float32
    dt = mybir.dt.bfloat16
    AF = mybir.ActivationFunctionType
    ALU = mybir.AluOpType

    nc = bass.Bass(num_devices=NCORES)

    xr = nc.declare_dram_parameter("xr", [2, 128, BPC, 11, 11], dt, isOutput=False)
    xl = nc.declare_dram_parameter("xl", [2, 128, BPC, 7, 7], dt, isOutput=False)
    w1t = nc.declare_dram_parameter("w1t", [2, 2, 128, 25, 128], dt, isOutput=False)
    w1s = nc.declare_dram_parameter("w1s", [2, 128, 2, 128], dt, isOutput=False)
    w2t = nc.declare_dram_parameter("w2t", [2, 2, 128, 9, 128], dt, isOutput=False)
    w3t = nc.declare_dram_parameter("w3t", [2, 2, 128, 9, 128], dt, isOutput=False)
    scal = nc.declare_dram_parameter("scal", [128, 14], f32, isOutput=False)
    pout_p = nc.declare_dram_parameter("pout", [128, 2 * BPC + 4], f32, isOutput=True)

    from contextlib import ExitStack
    NLANES = 8
    with ExitStack() as ctx:
        dma_sems = [ctx.enter_context(nc.semaphore(f"dma{j}")) for j in range(NLANES)]
        out_sem = ctx.enter_context(nc.semaphore("out_sem"))
        pe_sem = ctx.enter_context(nc.semaphore("pe_sem"))
        act_sem = ctx.enter_context(nc.semaphore("act_sem"))
        dve_sem = ctx.enter_context(nc.semaphore("dve_sem"))

        def sb(name, shape, d):
            return ctx.enter_context(nc.sbuf_tensor(name, shape, d))

        def pst(name, shape):
            return ctx.enter_context(nc.psum_tensor(name, shape, f32))

        scal_t = sb("scal_t", [128, 14], f32)
        scr0 = sb("scr0", [128, 1], f32)
        xl0, xl1 = sb("xl0", [128, BPC, 7, 7], dt), sb("xl1", [128, BPC, 7, 7], dt)
        w1s0, w1s1 = sb("w1s0", [128, 2, 128], dt), sb("w1s1", [128, 2, 128], dt)
        xr0, xr1 = sb("xr0", [128, BPC, 11, 11], dt), sb("xr1", [128, BPC, 11, 11], dt)
        w1_00, w1_10 = sb("w1_00", [128, 25, 128], dt), sb("w1_10", [128, 25, 128], dt)
        w1_01, w1_11 = sb("w1_01", [128, 25, 128], dt), sb("w1_11", [128, 25, 128], dt)
        w2_00, w2_10 = sb("w2_00", [128, 9, 128], dt), sb("w2_10", [128, 9, 128], dt)
        w2_01, w2_11 = sb("w2_01", [128, 9, 128], dt), sb("w2_11", [128, 9, 128], dt)
        w3_00, w3_10 = sb("w3_00", [128, 9, 128], dt), sb("w3_10", [128, 9, 128], dt)
        w3_01, w3_11 = sb("w3_01", [128, 9, 128], dt), sb("w3_11", [128, 9, 128], dt)
        r1_0, r1_1 = sb("r1_0", [128, BPC, 7, 7], dt), sb("r1_1", [128, BPC, 7, 7], dt)
        r2_0, r2_1 = sb("r2_0", [128, BPC, 5, 5], dt), sb("r2_1", [128, BPC, 5, 5], dt)
        y3_0, y3_1 = sb("y3_0", [128, BPC, 9], f32), sb("y3_1", [128, BPC, 9], f32)
        sq_scr = sb("sq_scr", [128, BPC, 9], f32)
        outsb = sb("outsb", [128, 2 * BPC + 4], f32)
        # one full 2KB bank per psum tensor (group + bank-safety isolation)
        psum_w = pst("psum_w", [128, 512])[:, 0:64]
        psum1_0 = pst("psum1_0", [128, 512])[:, 0:BPC * 49]
        psum1_1 = pst("psum1_1", [128, 512])[:, 0:BPC * 49]
        psum2_0 = pst("psum2_0", [128, 512])[:, 0:BPC * 25]
        psum2_1 = pst("psum2_1", [128, 512])[:, 0:BPC * 25]
        psum3_0 = pst("psum3_0", [128, 512])[:, 0:BPC * 9]
        psum3_1 = pst("psum3_1", [128, 512])[:, 0:BPC * 9]

        w1sb = [w1s0, w1s1]
        xlb = [xl0, xl1]
        xrb = [xr0, xr1]
        w1b = [[w1_00, w1_01], [w1_10, w1_11]]
        w2b = [[w2_00, w2_01], [w2_10, w2_11]]
        w3b = [[w3_00, w3_01], [w3_10, w3_11]]
        r1b, r2b, y3b = [r1_0, r1_1], [r2_0, r2_1], [y3_0, y3_1]
        psum1 = [psum1_0, psum1_1]
        psum2 = [psum2_0, psum2_1]
        psum3 = [psum3_0, psum3_1]
        partials = outsb[:, 2 * BPC:]
        ybar = [outsb[:, o * BPC:(o + 1) * BPC] for o in range(2)]

        # DMA completion ticks (1-based, each DMA adds 16)
        D = {}

        with nc.Block() as block:

            @block.sync
            def _(sync):
                k = 0
                lane_cnt = [0] * NLANES

                def dma(name, out, in_):
                    nonlocal k
                    lane = k % NLANES
                    # serialize same-lane DMAs through completion so the
                    # lane sem value is unambiguous (Tile's protocol)
                    if lane_cnt[lane] > 0:
                        sync.wait_ge(dma_sems[lane], 16 * lane_cnt[lane])
                    sync.dma_start(out=out, in_=in_).then_inc(dma_sems[lane], 16)
                    lane_cnt[lane] += 1
                    k += 1
                    D[name] = (lane, 16 * lane_cnt[lane])

                dma("scal", scal_t[:], scal[:])
                dma("xl0", xl0[:], xl[0])
                dma("w1s0", w1s0[:], w1s[0])
                dma("xl1", xl1[:], xl[1])
                dma("w1s1", w1s1[:], w1s[1])
                dma("xr0", xr0[:], xr[0])
                dma("w1_00a", w1_00[:, 0:13, :], w1t[0, 0, :, 0:13, :])
                dma("w1_00b", w1_00[:, 13:25, :], w1t[0, 0, :, 13:25, :])
                dma("xr1", xr1[:], xr[1])
                dma("w1_10a", w1_10[:, 0:13, :], w1t[1, 0, :, 0:13, :])
                dma("w1_10b", w1_10[:, 13:25, :], w1t[1, 0, :, 13:25, :])
                dma("w1_01a", w1_01[:, 0:13, :], w1t[0, 1, :, 0:13, :])
                dma("w1_01b", w1_01[:, 13:25, :], w1t[0, 1, :, 13:25, :])
                dma("w1_11a", w1_11[:, 0:13, :], w1t[1, 1, :, 0:13, :])
                dma("w1_11b", w1_11[:, 13:25, :], w1t[1, 1, :, 13:25, :])
                dma("w2_00", w2_00[:], w2t[0, 0])
                dma("w2_10", w2_10[:], w2t[1, 0])
                dma("w2_01", w2_01[:], w2t[0, 1])
                dma("w2_11", w2_11[:], w2t[1, 1])
                dma("w3_00", w3_00[:], w3t[0, 0])
                dma("w3_10", w3_10[:], w3t[1, 0])
                dma("w3_01", w3_01[:], w3t[0, 1])
                dma("w3_11", w3_11[:], w3t[1, 1])

            @block.tensor
            def _(pe):
                # warm-up while weights stream in (HAM to K=8/8)
                pe.wait_ge(dma_sems[D["w1s0"][0]], D["w1s0"][1])
                for _i in range(28):
                    pe.matmul(psum_w, w1s0[:, 0, :], w1s0[:, 0, 0:64],
                              start=True, stop=True)

                # conv1 (52 accumulating MMs per output chunk)
                for o in range(2):
                    first = (o == 0)
                    for nm in ("xl0", "w1s0", "xl1", "w1s1"):
                        pe.wait_ge(dma_sems[D[nm][0]], D[nm][1])
                    for i in range(2):
                        pe.matmul(psum1[o], w1sb[i][:, o, :], xlb[i][:],
                                  start=(i == 0), stop=False)
                    for i in range(2):
                        for h, taps in ((0, range(0, 13)), (1, range(13, 25))):
                            for nm in ((f"xr{i}", f"w1_{i}{o}a") if h == 0
                                       else (f"w1_{i}{o}b",)):
                                pe.wait_ge(dma_sems[D[nm][0]], D[nm][1])
                            for t in taps:
                                a, b = divmod(t, 5)
                                last = (i == 1 and t == 24)
                                mm = pe.matmul(psum1[o], w1b[i][o][:, t, :],
                                               xrb[i][:, :, a:a + 7, b:b + 7],
                                               start=False, stop=last)
                                if last:
                                    mm.then_inc(pe_sem, 1)

                # conv2 (r1 produced on DVE; wait per input half)
                for o in range(2):
                    for nm in (f"w2_0{o}", f"w2_1{o}"):
                        pe.wait_ge(dma_sems[D[nm][0]], D[nm][1])
                    k = 0
                    for i in range(2):
                        pe.wait_ge(dve_sem, 1 + i)
                        for t in range(9):
                            a, b = divmod(t, 3)
                            mm = pe.matmul(psum2[o], w2b[i][o][:, t, :],
                                           r1b[i][:, :, a:a + 5, b:b + 5],
                                           start=(k == 0), stop=(k == 17))
                            if k == 17:
                                mm.then_inc(pe_sem, 1)
                            k += 1

                # conv3
                for o in range(2):
                    for nm in (f"w3_0{o}", f"w3_1{o}"):
                        pe.wait_ge(dma_sems[D[nm][0]], D[nm][1])
                    k = 0
                    for i in range(2):
                        pe.wait_ge(dve_sem, 3 + i)
                        for t in range(9):
                            a, b = divmod(t, 3)
                            mm = pe.matmul(psum3[o], w3b[i][o][:, t, :],
                                           r2b[i][:, :, a:a + 3, b:b + 3],
                                           start=(k == 0), stop=(k == 17))
                            if k == 17:
                                mm.then_inc(pe_sem, 1)
                            k += 1

            @block.scalar
            def _(act):
                # touch scal early: preloads ACT table during the DMA window
                act.wait_ge(dma_sems[D["scal"][0]], D["scal"][1])
                act.activation(scr0[:], scal_t[:, 12:13], AF.Copy).then_inc(
                    act_sem, 1)
                for o in range(2):           # y3 = relu(psum3 + b3) + stats
                    act.wait_ge(pe_sem, 5 + o)
                    act.activation(y3b[o][:], psum3[o], AF.Relu,
                                   bias=scal_t[:, 4 + o:5 + o],
                                   accum_out=partials[:, o:o + 1]).then_inc(
                        act_sem, 1)
                    # self-wait: ACT pipelines, so the Square reading y3 must
                    # wait for the relu's completion tick
                    act.wait_ge(act_sem, 2 + 2 * o)
                    act.activation(sq_scr[:], y3b[o][:], AF.Square,
                                   accum_out=partials[:, 2 + o:3 + o]).then_inc(
                        act_sem, 1)

            @block.vector
            def _(dve):
                # r1/r2 relus on DVE: (psum + b) max 0, cast to bf16 -- faster
                # than ACT for the PE-critical path and runs on an idle engine
                for o in range(2):
                    dve.wait_ge(pe_sem, 1 + o)
                    dve.tensor_scalar(r1b[o][:], psum1[o],
                                      scal_t[:, 0 + o:1 + o], 0.0,
                                      ALU.add, ALU.max).then_inc(dve_sem, 1)
                for o in range(2):
                    dve.wait_ge(pe_sem, 3 + o)
                    dve.tensor_scalar(r2b[o][:], psum2[o],
                                      scal_t[:, 2 + o:3 + o], 0.0,
                                      ALU.add, ALU.max).then_inc(dve_sem, 1)
                for o in range(2):           # ybar = per-image spatial sum
                    dve.wait_ge(act_sem, 2 + 2 * o)   # y3[o] relu done
                    dve.tensor_reduce(ybar[o], y3b[o][:],
                                      axis=mybir.AxisListType.X,
                                      op=ALU.add).then_inc(dve_sem, 1)

            @block.gpsimd
            def _(gp):
                gp.wait_ge(act_sem, 5)
                gp.wait_ge(dve_sem, 6)
                gp.dma_start(out=pout_p[:], in_=outsb[:]).then_inc(out_sem, 16)
                gp.wait_ge(out_sem, 16)
                # (no sem_clear: NRT re-initializes semaphores per execution;
                # verified by the repeated-run correctness check in test.py)

    _split_multiwaits(nc, mybir)
    nc.finalize()
    return nc


def _np_dt(mode):
    if mode == "bf16":
        import ml_dtypes
        return ml_dtypes.bfloat16
    return np.float32


def _prep_inputs(inputs, mode):
    adt = _np_dt(mode)
    wdt = _np_dt(mode)

    x_r = np.asarray(inputs["x_r"], np.float32)
    x_l = np.asarray(inputs["x_l"], np.float32)
    w1 = np.asarray(inputs["w1"], np.float32)
    w2 = np.asarray(inputs["w2"], np.float32)
    w3 = np.asarray(inputs["w3"], np.float32)

    xp = np.pad(x_r, ((0, 0), (0, 0), (2, 2), (2, 2)))

    # lhsT layouts: [ci_chunk, co_chunk, ci_p, tap, co_p]
    w1t = np.ascontiguousarray(
        (-w1).transpose(1, 2, 3, 0).reshape(2, 128, 25, 2, 128)
        .transpose(0, 3, 1, 2, 4).astype(wdt))
    w1sum = np.ascontiguousarray(
        w1.sum(axis=(2, 3)).transpose(1, 0).reshape(2, 128, 2, 128).astype(wdt))
    w2t = np.ascontiguousarray(
        w2.transpose(1, 2, 3, 0).reshape(2, 128, 9, 2, 128)
        .transpose(0, 3, 1, 2, 4).astype(wdt))
    w3t = np.ascontiguousarray(
        w3.transpose(1, 2, 3, 0).reshape(2, 128, 9, 2, 128)
        .transpose(0, 3, 1, 2, 4).astype(wdt))

    scal = np.zeros((128, 14), np.float32)
    for col, name in ((0, "b1"), (2, "b2"), (4, "b3"), (6, "gamma"), (8, "beta")):
        scal[:, col:col + 2] = np.asarray(inputs[name], np.float32).reshape(2, 128).T
    scal[:, 10:12] = np.asarray(inputs["wl"], np.float32).reshape(2, 128).T
    scal[:, 12] = np.asarray(inputs["bl"], np.float32)[0]
    scal[:, 13] = BN_EPS

    in_maps = []
    for k in range(NCORES):
        sl = slice(k * BPC, (k + 1) * BPC)
        xr_k = np.ascontiguousarray(
            xp[sl].transpose(1, 0, 2, 3).reshape(2, 128, BPC, 11, 11).astype(adt))
        xl_k = np.ascontiguousarray(
            x_l[sl].transpose(1, 0, 2, 3).reshape(2, 128, BPC, 7, 7).astype(adt))
        in_maps.append({
            "xr": xr_k, "xl": xl_k,
            "w1t": w1t, "w1s": w1sum, "w2t": w2t, "w3t": w3t,
            "scal": scal,
        })
    return in_maps


def kernel(**inputs):
    global LAST_RESULT
    from concourse.bass_utils import run_bass_kernel_spmd

    mode, tail, impl = MM_MODE, TAIL, IMPL
    if impl == "raw" and (mode != "bf16" or tail != "host"):
        impl = "tile"
    key = (mode, tail, impl)
    if key not in _CACHE:
        _CACHE[key] = _build_raw(mode) if impl == "raw" else _build(mode, tail)
    nc = _CACHE[key]

    in_maps = _prep_inputs(inputs, mode)
    res = run_bass_kernel_spmd(nc, in_maps, list(range(NCORES)), trace=TRACE)
    LAST_RESULT = res

    return _postprocess(res.results, inputs, tail)


def _postprocess(results, inputs, tail):
    if tail == "cc":
        out = np.concatenate([r["out"] for r in results], axis=0)
        return out.astype(np.float32)

    # host-side unshard: combine per-core BN partials, apply affine + linear
    packed = np.stack([np.asarray(r["pout"], np.float32) for r in results])  # [8,128,20]
    ybar = np.stack([packed[:, :, 0:BPC], packed[:, :, BPC:2 * BPC]], axis=1)
    ybar = ybar.transpose(0, 1, 2, 3)                          # [8, 2, 128, 8]
    pout = packed[:, :, 2 * BPC:]                              # [8, 128, 4]
    tot = pout.sum(axis=0)                                     # [128, 4]
    n = float(B * 9)
    mean = (tot[:, 0:2] / n).T.reshape(C)                      # channel c = o*128+p
    q = (tot[:, 2:4] / n).T.reshape(C)
    var = q - mean * mean
    rstd = 1.0 / np.sqrt(var + BN_EPS)
    wl = np.asarray(inputs["wl"], np.float32).reshape(C)
    gamma = np.asarray(inputs["gamma"], np.float32).reshape(C)
    beta = np.asarray(inputs["beta"], np.float32).reshape(C)
    bl = np.asarray(inputs["bl"], np.float32).reshape(1)
    a0 = wl * gamma * rstd
    const = bl[0] + np.sum(wl * beta) - np.sum(a0 * mean)
    yb = ybar.transpose(0, 3, 1, 2).reshape(B, C)              # [64, 256] (c=o*128+p)
    out = (yb / 9.0) @ a0 + const
    return out.astype(np.float32).reshape(B, 1)
